# revision 14
# baseline (speedup 1.0000x reference)
"""Trainium2 Bass kernel for nn_CausalSelfAttention_2740189134905.

Self-contained: hardcodes shapes/sharding. Zero-collective, sequence-sharded
SPMD across 8 NeuronCores: core c owns query blocks {15-c, c} (128 rows each);
every core computes the full K/V projection (cheaper than on-chip collectives).
All matmuls run in float32r (fp32 with 11-bit mantissa, full PE rate at N>=256).
Per-core behavior differs only through input data (own xT columns, cos/sin
rows, additive causal masks), so one uniform program runs on all 8 cores.

Attention uses transposed scores ST[k, q] so P@V needs no per-tile transposes;
softmax (no max-subtraction; |scores| <= ~12 so exp is safe in fp32) sums via
a ones-column matmul on the PE; causal masks are host-built additive tiles
accumulated into the score PSUM via an identity-matmul.
"""

import os
from contextlib import ExitStack

import numpy as np

import concourse.bacc as bacc
import concourse.bass as bass
import concourse.mybir as mybir
import concourse.tile as tile
from concourse import masks as cmasks
from concourse.bass_utils import run_bass_kernel_spmd

FP32 = mybir.dt.float32
F32R = mybir.dt.float32r

H, KVH, HD, SHD = 16, 4, 128, 16
DIM = H * HD            # 2048
KV_DIM = KVH * HD       # 512
GATE_IN = 12
S = 2048
P = 128
NBLK = S // P           # 16
NCORES = 8
NB_B = 8                # slot-B uniform causal range (blocks 0..7 cover c<=7)
EPS = 1.1920928955078125e-07  # finfo(float32).eps
ISQ = float(1.0 / np.sqrt(HD))
HF = HD // 2            # 64
Add = mybir.AluOpType.add
Mult = mybir.AluOpType.mult
Sub = mybir.AluOpType.subtract
Maxop = mybir.AluOpType.max
AF = mybir.ActivationFunctionType
AX = mybir.AxisListType


def round_fp32r(x: np.ndarray) -> np.ndarray:
    x = np.ascontiguousarray(x, dtype=np.float32)
    u = x.view(np.uint32)
    r = (u + np.uint32(0x7FF) + ((u >> np.uint32(12)) & np.uint32(1))) & np.uint32(
        0xFFFFF000
    )
    return r.view(np.float32)


def _norm_rope(nc, pool, ps, nh, cos_j, sin_j, gains, dst, pool_div):
    """RMS-norm (+optional per-head gain) + rope + shared-dim pool for one
    [128, nh*HD] projection block sitting in PSUM. Writes dst [P, nh, HD]."""
    sq = pool.tile([P, nh * HD], FP32, tag="sq")
    nc.scalar.square(sq[:], ps[:])
    msq = pool.tile([P, nh], FP32, tag="msq")
    nc.vector.tensor_reduce(
        msq[:], sq[:].rearrange("p (h d) -> p h d", h=nh), axis=AX.X, op=Add
    )
    nc.vector.tensor_scalar(msq[:], msq[:], 1.0 / HD, EPS, op0=Mult, op1=Add)
    rr = pool.tile([P, nh], FP32, tag="rr")
    nc.vector.reciprocal(rr[:], msq[:])
    rs = pool.tile([P, nh], FP32, tag="rs")
    nc.scalar.sqrt(rs[:], rr[:])
    if gains is not None:
        nc.vector.tensor_tensor(rs[:], rs[:], gains, op=Mult)
    kn = pool.tile([P, nh, HD], FP32, tag="kn")
    rs_bc = rs[:].unsqueeze(2).broadcast_to((P, nh, HD))
    nc.vector.tensor_tensor(
        kn[:], ps[:].rearrange("p (h d) -> p h d", h=nh), rs_bc, op=Mult
    )
    cb = cos_j.unsqueeze(1).broadcast_to((P, nh, HF))
    sb = sin_j.unsqueeze(1).broadcast_to((P, nh, HF))
    t1 = pool.tile([P, nh, HF], FP32, tag="t1")
    t2 = pool.tile([P, nh, HF], FP32, tag="t2")
    nc.vector.tensor_tensor(t1[:], kn[:, :, 0:HF], cb, op=Mult)
    nc.vector.tensor_tensor(t2[:], kn[:, :, HF:HD], sb, op=Mult)
    nc.vector.tensor_tensor(dst[:, :, 0:HF], t1[:], t2[:], op=Add)
    nc.vector.tensor_tensor(t1[:], kn[:, :, HF:HD], cb, op=Mult)
    nc.vector.tensor_tensor(t2[:], kn[:, :, 0:HF], sb, op=Mult)
    nc.vector.tensor_tensor(dst[:, :, HF:HD], t1[:], t2[:], op=Sub)
    if pool_div is not None:
        pl = pool.tile([P, SHD], FP32, tag="pl")
        nc.vector.tensor_reduce(
            pl[:], dst[:, :, HD - SHD:HD].rearrange("p h d -> p d h"),
            axis=AX.X, op=Add,
        )
        plm = pool.tile([P, SHD], F32R, tag="plm")
        nc.vector.tensor_scalar(plm[:], pl[:], pool_div, 0.0, op0=Mult, op1=Add)
        nc.vector.tensor_copy(
            dst[:, :, HD - SHD:HD],
            plm[:].unsqueeze(1).broadcast_to((P, nh, SHD)),
        )


def build_program(repeat: int = 1, phase_limit: int = 99, p3sub: int = 99):
    nc = bacc.Bacc()
    dp = nc.declare_dram_parameter
    xT_d = dp("xT", [DIM, S], F32R, isOutput=False)
    xq_d = dp("xq", [DIM, 2 * P], F32R, isOutput=False)
    wkT_d = dp("wkT", [DIM, KV_DIM], F32R, isOutput=False)
    wvT_d = dp("wvT", [DIM, KV_DIM], F32R, isOutput=False)
    wqT_d = dp("wqT", [DIM, DIM], F32R, isOutput=False)
    wpT_d = dp("wpT", [DIM, DIM], F32R, isOutput=False)
    wgT_d = dp("wgT", [GATE_IN, H], F32R, isOutput=False)
    cosk_d = dp("cosk", [S, HF], FP32, isOutput=False)
    sink_d = dp("sink", [S, HF], FP32, isOutput=False)
    cosq_d = dp("cosq", [2 * P, HF], FP32, isOutput=False)
    sinq_d = dp("sinq", [2 * P, HF], FP32, isOutput=False)
    mask_d = dp("mask", [NBLK, P, 512], F32R, isOutput=False)
    gain_d = dp("gains", [1, H], FP32, isOutput=False)
    out_d = dp("out_rows", [2 * P, DIM], FP32, isOutput=True)
    vout_d = dp("v_out", [S, KV_DIM], F32R, isOutput=True)

    with tile.TileContext(nc) as tc, ExitStack() as ctx:
        const = ctx.enter_context(tc.tile_pool(name="const", bufs=1))
        ident_f = const.tile([P, P], FP32)
        cmasks.make_identity(nc, ident_f[:])
        ident = const.tile([P, P], F32R)
        nc.vector.tensor_copy(ident[:], ident_f[:])
        ones_f = const.tile([P, 1], FP32)
        nc.gpsimd.memset(ones_f[:], 1.0)
        ones_col = const.tile([P, 1], F32R)
        nc.vector.tensor_copy(ones_col[:], ones_f[:])
        one11 = const.tile([1, 1], FP32)
        nc.gpsimd.memset(one11[:], 1.0)
        wgT_sb = const.tile([GATE_IN, H], F32R)
        nc.sync.dma_start(wgT_sb[:], wgT_d[:])
        gain_sb = const.tile([1, H], FP32)
        nc.sync.dma_start(gain_sb[:], gain_d[:])
        gain_bc = const.tile([P, H], FP32)
        nc.gpsimd.partition_broadcast(gain_bc[:], gain_sb[:])

        big = ctx.enter_context(tc.tile_pool(name="big", bufs=1))
        kT = big.tile([P, KVH, S], F32R)           # 4MB  [hd, kvh, k]
        v_nat = big.tile([P, NBLK, KV_DIM], F32R)  # 4MB  [s%128, blk, kvh*hd]
        xq_sb = big.tile([P, NBLK, 2 * P], F32R)   # 2MB  [d%128, dtile, A|B]
        nc.sync.dma_start(xq_sb[:], xq_d.ap().rearrange("(t p) s -> p t s", p=P))
        qT = big.tile([P, 8, 512], F32R)           # 2MB [hd, pair, A0|A1|B0|B1]
        v_own = big.tile([P, 2, KV_DIM], FP32)     # [s, slot, kvh*hd]
        gate2 = big.tile([P, 2, H], FP32)          # [s, slot, h] = 2*sigmoid
        y_nat = big.tile([P, 2, H, HD], F32R)      # 2MB [s, slot, h, hd]

        for _ in range(repeat):
            # ---- Phase 1a: V projection (16 blocks + 2 own) ----
            with (
                tc.tile_pool(name="wv", bufs=1) as wv,
                tc.tile_pool(name="xsv", bufs=3) as xsv,
                tc.tile_pool(name="p1vps", bufs=3, space="PSUM") as p1vps,
            ):
                wv_sb = wv.tile([P, NBLK, KV_DIM], F32R)
                nc.sync.dma_start(
                    wv_sb[:], wvT_d.ap().rearrange("(t p) n -> p t n", p=P)
                )
                for j in range(NBLK):
                    xt = xsv.tile([P, NBLK, P], F32R, tag="xt")
                    nc.sync.dma_start(
                        xt[:],
                        xT_d[:, j * P:(j + 1) * P].rearrange("(t p) s -> p t s", p=P),
                    )
                    vps = p1vps.tile([P, KV_DIM], FP32, tag="vps")
                    for t in range(NBLK):
                        nc.tensor.matmul(
                            vps[:], xt[:, t, :], wv_sb[:, t, :],
                            start=(t == 0), stop=(t == NBLK - 1),
                        )
                    nc.vector.tensor_copy(v_nat[:, j, :], vps[:])
                    nc.sync.dma_start(vout_d[j * P:(j + 1) * P, :], v_nat[:, j, :])
                for s in range(2):
                    vops = p1vps.tile([P, KV_DIM], FP32, tag="vps")
                    for t in range(NBLK):
                        nc.tensor.matmul(
                            vops[:], xq_sb[:, t, s * P:(s + 1) * P], wv_sb[:, t, :],
                            start=(t == 0), stop=(t == NBLK - 1),
                        )
                    nc.vector.tensor_copy(v_own[:, s, :], vops[:])

            if phase_limit < 2:
                continue
            # ---- Phase 1b: K projection + norm/rope/pool + transpose ----
            with (
                tc.tile_pool(name="wk", bufs=1) as wk,
                tc.tile_pool(name="xsk", bufs=3) as xsk,
                tc.tile_pool(name="csk", bufs=1) as csk,
                tc.tile_pool(name="p1w", bufs=2) as p1w,
                tc.tile_pool(name="p1kps", bufs=2, space="PSUM") as p1kps,
                tc.tile_pool(name="p1tr", bufs=3, space="PSUM") as p1tr,
            ):
                wk_sb = wk.tile([P, NBLK, KV_DIM], F32R)
                nc.sync.dma_start(
                    wk_sb[:], wkT_d.ap().rearrange("(t p) n -> p t n", p=P)
                )
                cosk_sb = csk.tile([P, NBLK, HF], FP32)
                sink_sb = csk.tile([P, NBLK, HF], FP32)
                nc.sync.dma_start(
                    cosk_sb[:], cosk_d.ap().rearrange("(j p) c -> p j c", p=P)
                )
                nc.sync.dma_start(
                    sink_sb[:], sink_d.ap().rearrange("(j p) c -> p j c", p=P)
                )
                for j in range(NBLK):
                    xt = xsk.tile([P, NBLK, P], F32R, tag="xt")
                    nc.sync.dma_start(
                        xt[:],
                        xT_d[:, j * P:(j + 1) * P].rearrange("(t p) s -> p t s", p=P),
                    )
                    kps = p1kps.tile([P, KV_DIM], FP32, tag="kps")
                    for t in range(NBLK):
                        nc.tensor.matmul(
                            kps[:], xt[:, t, :], wk_sb[:, t, :],
                            start=(t == 0), stop=(t == NBLK - 1),
                        )
                    k_ro = p1w.tile([P, KVH, HD], F32R, tag="kro")
                    _norm_rope(
                        nc, p1w, kps, KVH, cosk_sb[:, j, :], sink_sb[:, j, :],
                        None, k_ro, 1.0 / KVH,
                    )
                    for g in range(KVH):
                        tp = p1tr.tile([P, P], F32R, tag="tp")
                        nc.tensor.transpose(tp[:], k_ro[:, g, :], ident[:])
                        nc.vector.tensor_copy(kT[:, g, j * P:(j + 1) * P], tp[:])

            if phase_limit < 3:
                continue
            # ---- Phase 2: Q projection + gate + qT ----
            with (
                tc.tile_pool(name="csq", bufs=1) as csq,
                tc.tile_pool(name="wqs", bufs=6) as wqs,
                tc.tile_pool(name="p2w", bufs=2) as p2w,
                tc.tile_pool(name="qro", bufs=1) as qro,
                tc.tile_pool(name="p2ps", bufs=2, space="PSUM") as p2ps,
                tc.tile_pool(name="p2tr", bufs=2, space="PSUM") as p2tr,
            ):
                cosq_sb = csq.tile([P, 2, HF], FP32)
                sinq_sb = csq.tile([P, 2, HF], FP32)
                nc.sync.dma_start(
                    cosq_sb[:], cosq_d.ap().rearrange("(s p) c -> p s c", p=P)
                )
                nc.sync.dma_start(
                    sinq_sb[:], sinq_d.ap().rearrange("(s p) c -> p s c", p=P)
                )
                q_ro = qro.tile([P, 2, H, HD], F32R)
                for ch in range(4):
                    wq_t = []
                    for t in range(NBLK):
                        wt = wqs.tile([P, 512], F32R, tag="wq")
                        nc.sync.dma_start(
                            wt[:], wqT_d[t * P:(t + 1) * P, ch * 512:(ch + 1) * 512]
                        )
                        wq_t.append(wt)
                    for s in range(2):
                        qps = p2ps.tile([P, 512], FP32, tag="qps")
                        for t in range(NBLK):
                            nc.tensor.matmul(
                                qps[:], xq_sb[:, t, s * P:(s + 1) * P], wq_t[t][:],
                                start=(t == 0), stop=(t == NBLK - 1),
                            )
                        _norm_rope(
                            nc, p2w, qps, 4, cosq_sb[:, s, :], sinq_sb[:, s, :],
                            gain_bc[:, ch * 4:(ch + 1) * 4],
                            q_ro[:, s, ch * 4:(ch + 1) * 4, :], None,
                        )
                for s in range(2):
                    pl = p2w.tile([P, SHD], FP32, tag="plq")
                    nc.vector.tensor_reduce(
                        pl[:],
                        q_ro[:, s, :, HD - SHD:HD].rearrange("p h d -> p d h"),
                        axis=AX.X, op=Add,
                    )
                    plm = p2w.tile([P, SHD], F32R, tag="plmq")
                    nc.vector.tensor_scalar(
                        plm[:], pl[:], 1.0 / H, 0.0, op0=Mult, op1=Add
                    )
                    nc.vector.tensor_copy(
                        q_ro[:, s, :, HD - SHD:HD],
                        plm[:].unsqueeze(1).broadcast_to((P, H, SHD)),
                    )
                    gps = p2ps.tile([P, H], FP32, tag="gate")
                    nc.tensor.matmul(
                        gps[:], xq_sb[0:GATE_IN, 0, s * P:(s + 1) * P], wgT_sb[:],
                        start=True, stop=True,
                    )
                    nc.scalar.activation(gate2[:, s, :], gps[:], AF.Sigmoid)
                    nc.vector.tensor_scalar(
                        gate2[:, s, :], gate2[:, s, :], 2.0, 0.0, op0=Mult, op1=Add
                    )
                    for h in range(H):
                        tp = p2tr.tile([P, P], F32R, tag="tp")
                        nc.tensor.transpose(tp[:], q_ro[:, s, h, :], ident[:])
                        pair, off = h // 2, s * 256 + (h % 2) * P
                        nc.scalar.activation(
                            qT[:, pair, off:off + P], tp[:], AF.Copy, scale=ISQ
                        )

            if phase_limit < 4:
                continue
            # ---- Phase 3: attention per kv-group ----
            with (
                tc.tile_pool(name="msk", bufs=1) as msk,
                tc.tile_pool(name="exw", bufs=3) as exw,
                tc.tile_pool(name="p3w", bufs=2) as p3w,
                tc.tile_pool(name="stps", bufs=2, space="PSUM") as stps,
                tc.tile_pool(name="ytps", bufs=2, space="PSUM") as ytps,
                tc.tile_pool(name="dnps", bufs=2, space="PSUM") as dnps,
                tc.tile_pool(name="p3tr", bufs=1, space="PSUM") as p3tr,
            ):
                mask_sb = msk.tile([P, NBLK, 512], F32R)
                nc.sync.dma_start(mask_sb[:], mask_d.ap().rearrange("t p n -> p t n"))
                for g in range(KVH):
                    pairs = (2 * g, 2 * g + 1)
                    yt_ps = {pi: ytps.tile([P, 512], FP32, name="ytp", tag="ytp") for pi in pairs}
                    den_ps = {pi: dnps.tile([1, 512], FP32, name="dnp", tag="dnp") for pi in pairs}
                    for t in range(NBLK):
                        w = 512 if t < NB_B else 256
                        for pi in pairs:
                            st = stps.tile([P, 512], FP32, tag="st")
                            nc.tensor.matmul(
                                st[:, 0:w], kT[:, g, t * P:(t + 1) * P],
                                qT[:, pi, 0:w], start=True, stop=False,
                            )
                            nc.tensor.matmul(
                                st[:, 0:w], ident[:], mask_sb[:, t, 0:w],
                                start=False, stop=True,
                            )
                            ex = exw.tile([P, 512], F32R, tag="ex")
                            nc.scalar.activation(ex[:, 0:w], st[:, 0:w], AF.Exp)
                            if p3sub < 2:
                                continue
                            nc.tensor.matmul(
                                den_ps[pi][:, 0:w], ones_col[:], ex[:, 0:w],
                                start=(t == 0), stop=(t == NBLK - 1),
                                skip_group_check=True,
                            )
                            nc.tensor.matmul(
                                yt_ps[pi][:, 0:w],
                                v_nat[:, t, g * HD:(g + 1) * HD], ex[:, 0:w],
                                start=(t == 0), stop=(t == NBLK - 1),
                                skip_group_check=True,
                            )
                    if p3sub < 3:
                        continue
                    for pi in pairs:
                        den_sb = p3w.tile([1, 512], FP32, tag="densb")
                        nc.vector.tensor_copy(den_sb[:], den_ps[pi][:])
                        dtp = p3tr.tile([P, 4], FP32, tag="dtp")
                        for sub in range(4):
                            nc.tensor.matmul(
                                dtp[:, sub:sub + 1],
                                den_sb[0:1, sub * P:(sub + 1) * P], one11[:],
                                start=True, stop=True, skip_group_check=True,
                            )
                        rdn = p3w.tile([P, 4], FP32, tag="rdn")
                        nc.vector.reciprocal(rdn[:], dtp[:])
                        sc4 = p3w.tile([P, 4], FP32, tag="sc4")
                        nc.vector.tensor_tensor(
                            sc4[:], rdn[:], gate2[:, :, 2 * pi:2 * pi + 2], op=Mult
                        )
                        if p3sub < 4:
                            continue
                        yt_sb = p3w.tile([P, 512], F32R, tag="ytsb")
                        nc.vector.tensor_copy(yt_sb[:], yt_ps[pi][:])
                        for sub in range(4):
                            slot, h = sub // 2, 2 * pi + (sub % 2)
                            typs = p3tr.tile([P, P], F32R, tag="typs")
                            nc.tensor.transpose(
                                typs[:], yt_sb[:, sub * P:(sub + 1) * P], ident[:]
                            )
                            nc.scalar.activation(
                                y_nat[:, slot, h, :], typs[:], AF.Copy,
                                scale=sc4[:, sub:sub + 1],
                            )
                    if p3sub < 5:
                        continue
                    for s in range(2):
                        vg = v_own[:, s, g * HD:(g + 1) * HD]
                        scr = p3w.tile([P, HD], FP32, tag="scr")
                        nc.vector.tensor_tensor(scr[:], vg, vg, op=Mult)
                        n2 = p3w.tile([P, 1], FP32, tag="n2")
                        nc.vector.tensor_reduce(n2[:], scr[:], axis=AX.X, op=Add)
                        nc.vector.tensor_scalar(
                            n2[:], n2[:], 1.0, 1e-24, op0=Mult, op1=Add
                        )
                        rcp2 = p3w.tile([P, 1], FP32, tag="rcp2")
                        nc.vector.reciprocal(rcp2[:], n2[:])
                        rcp2n = p3w.tile([P, 1], FP32, tag="rcp2n")
                        nc.vector.tensor_scalar(
                            rcp2n[:], rcp2[:], -1.0, 0.0, op0=Mult, op1=Add
                        )
                        for h in range(4 * g, 4 * g + 4):
                            yf = y_nat[:, s, h, :].bitcast(FP32)
                            scr2 = p3w.tile([P, HD], FP32, tag="scr2")
                            nc.vector.tensor_tensor(scr2[:], yf, vg, op=Mult)
                            dot = p3w.tile([P, 1], FP32, tag="dot")
                            nc.vector.tensor_reduce(
                                dot[:], scr2[:], axis=AX.X, op=Add
                            )
                            cfn = p3w.tile([P, 1], FP32, tag="cfn")
                            nc.vector.tensor_tensor(
                                cfn[:], dot[:], rcp2n[:], op=Mult
                            )
                            vc = p3w.tile([P, HD], FP32, tag="vc")
                            nc.scalar.activation(
                                vc[:], vg, AF.Copy, scale=cfn[:]
                            )
                            yc = p3w.tile([P, HD], FP32, tag="yc")
                            nc.vector.tensor_tensor(yc[:], yf, vc[:], op=Add)
                            nc.vector.tensor_copy(y_nat[:, s, h, :], yc[:])
            # ---- Phase 4: output projection ----
            with (
                tc.tile_pool(name="wps", bufs=6) as wps,
                tc.tile_pool(name="ytf", bufs=1) as ytf,
                tc.tile_pool(name="p4w", bufs=2) as p4w,
                tc.tile_pool(name="p4ps", bufs=2, space="PSUM") as p4ps,
                tc.tile_pool(name="p4tr", bufs=2, space="PSUM") as p4tr,
            ):
                yT_fin = ytf.tile([P, H, 2 * P], F32R, tag="ytfin")
                for s in range(2):
                    for h in range(H):
                        tp = p4tr.tile([P, P], F32R, tag="tp")
                        nc.tensor.transpose(tp[:], y_nat[:, s, h, :], ident[:])
                        nc.vector.tensor_copy(yT_fin[:, h, s * P:(s + 1) * P], tp[:])
                for ch in range(4):
                    wp_t = []
                    for t in range(NBLK):
                        wt = wps.tile([P, 512], F32R, tag="wp")
                        nc.sync.dma_start(
                            wt[:], wpT_d[t * P:(t + 1) * P, ch * 512:(ch + 1) * 512]
                        )
                        wp_t.append(wt)
                    for s in range(2):
                        ops = p4ps.tile([P, 512], FP32, tag="ops")
                        for t in range(NBLK):
                            nc.tensor.matmul(
                                ops[:], yT_fin[:, t, s * P:(s + 1) * P], wp_t[t][:],
                                start=(t == 0), stop=(t == NBLK - 1),
                            )
                        o_sb = p4w.tile([P, 512], FP32, tag="osb")
                        nc.vector.tensor_copy(o_sb[:], ops[:])
                        nc.sync.dma_start(
                            out_d[s * P:(s + 1) * P, ch * 512:(ch + 1) * 512], o_sb[:]
                        )
    nc.compile()
    return nc


_prog_cache = {}


def _get_program(repeat: int):
    if repeat not in _prog_cache:
        _prog_cache[repeat] = build_program(repeat)
    return _prog_cache[repeat]


def _host_prep(x, cos, sin, Wq, Wk, Wv, Wproj, q_gain, Wg):
    x2 = np.ascontiguousarray(np.asarray(x, np.float32).reshape(S, DIM))
    xT = round_fp32r(x2.T)
    wkT = round_fp32r(np.asarray(Wk, np.float32).T)
    wvT = round_fp32r(np.asarray(Wv, np.float32).T)
    wqT = round_fp32r(np.asarray(Wq, np.float32).T)
    wpT = round_fp32r(np.asarray(Wproj, np.float32).T)
    wgT = round_fp32r(np.asarray(Wg, np.float32).T)
    cosf = np.ascontiguousarray(np.asarray(cos, np.float32).reshape(S, HF))
    sinf = np.ascontiguousarray(np.asarray(sin, np.float32).reshape(S, HF))
    gains = np.ascontiguousarray(
        np.asarray(q_gain, np.float32).reshape(1, H)
    )

    tri_keep = np.triu(np.ones((P, P), bool))  # keep where col >= row
    NEG = np.float32(-1e30)

    in_maps = []
    for c in range(NCORES):
        a, b = 15 - c, c
        rowsA = slice(a * P, (a + 1) * P)
        rowsB = slice(b * P, (b + 1) * P)
        xq = np.concatenate([xT[:, rowsA], xT[:, rowsB]], axis=1)
        cosq = np.concatenate([cosf[rowsA], cosf[rowsB]], axis=0)
        sinq = np.concatenate([sinf[rowsA], sinf[rowsB]], axis=0)
        mask = np.zeros((NBLK, P, 512), np.float32)
        for t in range(NBLK):
            mA = np.zeros((P, P), np.float32)
            if t == a:
                mA[~tri_keep] = NEG
            elif t > a:
                mA[:] = NEG
            mB = np.zeros((P, P), np.float32)
            if t == b:
                mB[~tri_keep] = NEG
            elif t > b:
                mB[:] = NEG
            mask[t] = np.concatenate([mA, mA, mB, mB], axis=1)
        in_maps.append({
            "xT": xT, "xq": np.ascontiguousarray(xq),
            "wkT": wkT, "wvT": wvT, "wqT": wqT, "wpT": wpT, "wgT": wgT,
            "cosk": cosf, "sink": sinf,
            "cosq": np.ascontiguousarray(cosq),
            "sinq": np.ascontiguousarray(sinq),
            "mask": round_fp32r(mask), "gains": gains,
        })
    return in_maps


def kernel(x, cos, sin, Wq, Wk, Wv, Wproj, q_gain, Wg, _repeat=None):
    repeat = _repeat or int(os.environ.get("KERNEL_REPEAT", "1"))
    nc = _get_program(repeat)
    in_maps = _host_prep(x, cos, sin, Wq, Wk, Wv, Wproj, q_gain, Wg)
    res = run_bass_kernel_spmd(nc, in_maps, list(range(NCORES)))
    out = np.zeros((S, DIM), np.float32)
    for c in range(NCORES):
        r = res.results[c]["out_rows"]
        out[(15 - c) * P:(16 - c) * P] = r[0:P]
        out[c * P:(c + 1) * P] = r[P:2 * P]
    v = np.ascontiguousarray(res.results[0]["v_out"]).reshape(1, S, KVH, HD)
    return out.reshape(1, S, DIM), v


# revision 15
# speedup vs baseline: 157.9988x; 157.9988x over previous
"""Trainium2 Bass kernel for nn_CausalSelfAttention_2740189134905.

Self-contained: hardcodes shapes/sharding. Zero-collective, sequence-sharded
SPMD across 8 NeuronCores: core c owns query blocks {15-c, c} (128 rows each);
every core computes the full K/V projection (cheaper than on-chip collectives).
All matmuls run in float32r (fp32 with 11-bit mantissa, full PE rate at N>=256).
Per-core behavior differs only through input data (own xT columns, cos/sin
rows, additive causal masks), so one uniform program runs on all 8 cores.

Attention uses transposed scores ST[k, q] so P@V needs no per-tile transposes;
softmax (no max-subtraction; |scores| <= ~12 so exp is safe in fp32) sums via
a ones-column matmul on the PE; causal masks are host-built additive tiles
accumulated into the score PSUM via an identity-matmul.
"""

import os
from contextlib import ExitStack

import numpy as np

import concourse.bacc as bacc
import concourse.bass as bass
import concourse.mybir as mybir
import concourse.tile as tile
from concourse import masks as cmasks
from concourse.bass_utils import run_bass_kernel_spmd

FP32 = mybir.dt.float32
F32R = mybir.dt.float32r

H, KVH, HD, SHD = 16, 4, 128, 16
DIM = H * HD            # 2048
KV_DIM = KVH * HD       # 512
GATE_IN = 12
S = 2048
P = 128
NBLK = S // P           # 16
NCORES = 8
NB_B = 8                # slot-B uniform causal range (blocks 0..7 cover c<=7)
EPS = 1.1920928955078125e-07  # finfo(float32).eps
ISQ = float(1.0 / np.sqrt(HD))
HF = HD // 2            # 64
Add = mybir.AluOpType.add
Mult = mybir.AluOpType.mult
Sub = mybir.AluOpType.subtract
Maxop = mybir.AluOpType.max
AF = mybir.ActivationFunctionType
AX = mybir.AxisListType


def round_fp32r(x: np.ndarray) -> np.ndarray:
    x = np.ascontiguousarray(x, dtype=np.float32)
    u = x.view(np.uint32)
    r = (u + np.uint32(0x7FF) + ((u >> np.uint32(12)) & np.uint32(1))) & np.uint32(
        0xFFFFF000
    )
    return r.view(np.float32)


def _norm_rope(nc, pool, ps, nh, cos_j, sin_j, gains, dst, pool_div):
    """RMS-norm (+optional per-head gain) + rope + shared-dim pool for one
    [128, nh*HD] projection block sitting in PSUM. Writes dst [P, nh, HD]."""
    sq = pool.tile([P, nh * HD], FP32, tag="sq")
    nc.scalar.square(sq[:], ps[:])
    msq = pool.tile([P, nh], FP32, tag="msq")
    nc.vector.tensor_reduce(
        msq[:], sq[:].rearrange("p (h d) -> p h d", h=nh), axis=AX.X, op=Add
    )
    nc.vector.tensor_scalar(msq[:], msq[:], 1.0 / HD, EPS, op0=Mult, op1=Add)
    rr = pool.tile([P, nh], FP32, tag="rr")
    nc.vector.reciprocal(rr[:], msq[:])
    rs = pool.tile([P, nh], FP32, tag="rs")
    nc.scalar.sqrt(rs[:], rr[:])
    if gains is not None:
        nc.vector.tensor_tensor(rs[:], rs[:], gains, op=Mult)
    kn = pool.tile([P, nh, HD], FP32, tag="kn")
    rs_bc = rs[:].unsqueeze(2).broadcast_to((P, nh, HD))
    nc.vector.tensor_tensor(
        kn[:], ps[:].rearrange("p (h d) -> p h d", h=nh), rs_bc, op=Mult
    )
    cb = cos_j.unsqueeze(1).broadcast_to((P, nh, HF))
    sb = sin_j.unsqueeze(1).broadcast_to((P, nh, HF))
    t1 = pool.tile([P, nh, HF], FP32, tag="t1")
    t2 = pool.tile([P, nh, HF], FP32, tag="t2")
    nc.vector.tensor_tensor(t1[:], kn[:, :, 0:HF], cb, op=Mult)
    nc.vector.tensor_tensor(t2[:], kn[:, :, HF:HD], sb, op=Mult)
    nc.vector.tensor_tensor(dst[:, :, 0:HF], t1[:], t2[:], op=Add)
    nc.vector.tensor_tensor(t1[:], kn[:, :, HF:HD], cb, op=Mult)
    nc.vector.tensor_tensor(t2[:], kn[:, :, 0:HF], sb, op=Mult)
    nc.vector.tensor_tensor(dst[:, :, HF:HD], t1[:], t2[:], op=Sub)
    if pool_div is not None:
        pl = pool.tile([P, SHD], FP32, tag="pl")
        nc.vector.tensor_reduce(
            pl[:], dst[:, :, HD - SHD:HD].rearrange("p h d -> p d h"),
            axis=AX.X, op=Add,
        )
        plm = pool.tile([P, SHD], F32R, tag="plm")
        nc.vector.tensor_scalar(plm[:], pl[:], pool_div, 0.0, op0=Mult, op1=Add)
        nc.vector.tensor_copy(
            dst[:, :, HD - SHD:HD],
            plm[:].unsqueeze(1).broadcast_to((P, nh, SHD)),
        )


def build_program(repeat: int = 1, phase_limit: int = 99, p3sub: int = 99):
    nc = bacc.Bacc()
    dp = nc.declare_dram_parameter
    xT_d = dp("xT", [DIM, S], F32R, isOutput=False)
    xq_d = dp("xq", [DIM, 2 * P], F32R, isOutput=False)
    wkT_d = dp("wkT", [DIM, KV_DIM], F32R, isOutput=False)
    wvT_d = dp("wvT", [DIM, KV_DIM], F32R, isOutput=False)
    wqT_d = dp("wqT", [DIM, DIM], F32R, isOutput=False)
    wpT_d = dp("wpT", [DIM, DIM], F32R, isOutput=False)
    wgT_d = dp("wgT", [GATE_IN, H], F32R, isOutput=False)
    cosk_d = dp("cosk", [S, HF], FP32, isOutput=False)
    sink_d = dp("sink", [S, HF], FP32, isOutput=False)
    cosq_d = dp("cosq", [2 * P, HF], FP32, isOutput=False)
    sinq_d = dp("sinq", [2 * P, HF], FP32, isOutput=False)
    mask_d = dp("mask", [NBLK, P, 512], F32R, isOutput=False)
    gain_d = dp("gains", [1, H], FP32, isOutput=False)
    out_d = dp("out_rows", [2 * P, DIM], FP32, isOutput=True)
    vout_d = dp("v_out", [S, KV_DIM], F32R, isOutput=True)

    with tile.TileContext(nc) as tc, ExitStack() as ctx:
        const = ctx.enter_context(tc.tile_pool(name="const", bufs=1))
        ident_f = const.tile([P, P], FP32)
        cmasks.make_identity(nc, ident_f[:])
        ident = const.tile([P, P], F32R)
        nc.vector.tensor_copy(ident[:], ident_f[:])
        ones_f = const.tile([P, 1], FP32)
        nc.gpsimd.memset(ones_f[:], 1.0)
        ones_col = const.tile([P, 1], F32R)
        nc.vector.tensor_copy(ones_col[:], ones_f[:])
        one11 = const.tile([1, 1], FP32)
        nc.gpsimd.memset(one11[:], 1.0)
        wgT_sb = const.tile([GATE_IN, H], F32R)
        nc.sync.dma_start(wgT_sb[:], wgT_d[:])
        gain_sb = const.tile([1, H], FP32)
        nc.sync.dma_start(gain_sb[:], gain_d[:])
        gain_bc = const.tile([P, H], FP32)
        nc.gpsimd.partition_broadcast(gain_bc[:], gain_sb[:])

        big = ctx.enter_context(tc.tile_pool(name="big", bufs=1))
        kT = big.tile([P, KVH, S], F32R)           # 4MB  [hd, kvh, k]
        v_nat = big.tile([P, NBLK, KV_DIM], F32R)  # 4MB  [s%128, blk, kvh*hd]
        xq_sb = big.tile([P, NBLK, 2 * P], F32R)   # 2MB  [d%128, dtile, A|B]
        nc.sync.dma_start(xq_sb[:], xq_d.ap().rearrange("(t p) s -> p t s", p=P))
        qT = big.tile([P, 8, 512], F32R)           # 2MB [hd, pair, A0|A1|B0|B1]
        v_own = big.tile([P, 2, KV_DIM], FP32)     # [s, slot, kvh*hd]
        gate2 = big.tile([P, 2, H], FP32)          # [s, slot, h] = 2*sigmoid
        y_nat = big.tile([P, 2, H, HD], F32R)      # 2MB [s, slot, h, hd]

        for _ in range(repeat):
            # ---- Phase 1a: V projection (16 blocks + 2 own) ----
            with (
                tc.tile_pool(name="wv", bufs=1) as wv,
                tc.tile_pool(name="xsv", bufs=4) as xsv,
                tc.tile_pool(name="p1vps", bufs=4, space="PSUM") as p1vps,
            ):
                wv_sb = wv.tile([P, NBLK, KV_DIM], F32R)
                nc.sync.dma_start(
                    wv_sb[:], wvT_d.ap().rearrange("(t p) n -> p t n", p=P)
                )
                for j in range(NBLK):
                    xt = xsv.tile([P, NBLK, P], F32R, tag="xt")
                    nc.sync.dma_start(
                        xt[:],
                        xT_d[:, j * P:(j + 1) * P].rearrange("(t p) s -> p t s", p=P),
                    )
                    vps = p1vps.tile([P, KV_DIM], FP32, tag="vps")
                    for t in range(NBLK):
                        nc.tensor.matmul(
                            vps[:], xt[:, t, :], wv_sb[:, t, :],
                            start=(t == 0), stop=(t == NBLK - 1),
                        )
                    nc.vector.tensor_copy(v_nat[:, j, :], vps[:])
                    nc.sync.dma_start(vout_d[j * P:(j + 1) * P, :], v_nat[:, j, :])
                for s in range(2):
                    vops = p1vps.tile([P, KV_DIM], FP32, tag="vps")
                    for t in range(NBLK):
                        nc.tensor.matmul(
                            vops[:], xq_sb[:, t, s * P:(s + 1) * P], wv_sb[:, t, :],
                            start=(t == 0), stop=(t == NBLK - 1),
                        )
                    nc.vector.tensor_copy(v_own[:, s, :], vops[:])

            if phase_limit < 2:
                continue
            # ---- Phase 1b: K projection + norm/rope/pool + transpose ----
            with (
                tc.tile_pool(name="wk", bufs=1) as wk,
                tc.tile_pool(name="xsk", bufs=4) as xsk,
                tc.tile_pool(name="csk", bufs=1) as csk,
                tc.tile_pool(name="p1w", bufs=2) as p1w,
                tc.tile_pool(name="p1kps", bufs=3, space="PSUM") as p1kps,
                tc.tile_pool(name="p1tr", bufs=3, space="PSUM") as p1tr,
            ):
                wk_sb = wk.tile([P, NBLK, KV_DIM], F32R)
                nc.sync.dma_start(
                    wk_sb[:], wkT_d.ap().rearrange("(t p) n -> p t n", p=P)
                )
                cosk_sb = csk.tile([P, NBLK, HF], FP32)
                sink_sb = csk.tile([P, NBLK, HF], FP32)
                nc.sync.dma_start(
                    cosk_sb[:], cosk_d.ap().rearrange("(j p) c -> p j c", p=P)
                )
                nc.sync.dma_start(
                    sink_sb[:], sink_d.ap().rearrange("(j p) c -> p j c", p=P)
                )
                for j in range(NBLK):
                    xt = xsk.tile([P, NBLK, P], F32R, tag="xt")
                    nc.sync.dma_start(
                        xt[:],
                        xT_d[:, j * P:(j + 1) * P].rearrange("(t p) s -> p t s", p=P),
                    )
                    kps = p1kps.tile([P, KV_DIM], FP32, tag="kps")
                    for t in range(NBLK):
                        nc.tensor.matmul(
                            kps[:], xt[:, t, :], wk_sb[:, t, :],
                            start=(t == 0), stop=(t == NBLK - 1),
                        )
                    k_ro = p1w.tile([P, KVH, HD], F32R, tag="kro")
                    _norm_rope(
                        nc, p1w, kps, KVH, cosk_sb[:, j, :], sink_sb[:, j, :],
                        None, k_ro, 1.0 / KVH,
                    )
                    for g in range(KVH):
                        tp = p1tr.tile([P, P], F32R, tag="tp")
                        nc.tensor.transpose(tp[:], k_ro[:, g, :], ident[:])
                        nc.vector.tensor_copy(kT[:, g, j * P:(j + 1) * P], tp[:])

            if phase_limit < 3:
                continue
            # ---- Phase 2: Q projection + gate + qT ----
            with (
                tc.tile_pool(name="csq", bufs=1) as csq,
                tc.tile_pool(name="wqs", bufs=8) as wqs,
                tc.tile_pool(name="p2w", bufs=2) as p2w,
                tc.tile_pool(name="qro", bufs=1) as qro,
                tc.tile_pool(name="p2ps", bufs=2, space="PSUM") as p2ps,
                tc.tile_pool(name="p2tr", bufs=2, space="PSUM") as p2tr,
            ):
                cosq_sb = csq.tile([P, 2, HF], FP32)
                sinq_sb = csq.tile([P, 2, HF], FP32)
                nc.sync.dma_start(
                    cosq_sb[:], cosq_d.ap().rearrange("(s p) c -> p s c", p=P)
                )
                nc.sync.dma_start(
                    sinq_sb[:], sinq_d.ap().rearrange("(s p) c -> p s c", p=P)
                )
                q_ro = qro.tile([P, 2, H, HD], F32R)
                for ch in range(4):
                    wq_t = []
                    for t in range(NBLK):
                        wt = wqs.tile([P, 512], F32R, tag="wq")
                        nc.sync.dma_start(
                            wt[:], wqT_d[t * P:(t + 1) * P, ch * 512:(ch + 1) * 512]
                        )
                        wq_t.append(wt)
                    for s in range(2):
                        qps = p2ps.tile([P, 512], FP32, tag="qps")
                        for t in range(NBLK):
                            nc.tensor.matmul(
                                qps[:], xq_sb[:, t, s * P:(s + 1) * P], wq_t[t][:],
                                start=(t == 0), stop=(t == NBLK - 1),
                            )
                        _norm_rope(
                            nc, p2w, qps, 4, cosq_sb[:, s, :], sinq_sb[:, s, :],
                            gain_bc[:, ch * 4:(ch + 1) * 4],
                            q_ro[:, s, ch * 4:(ch + 1) * 4, :], None,
                        )
                for s in range(2):
                    pl = p2w.tile([P, SHD], FP32, tag="plq")
                    nc.vector.tensor_reduce(
                        pl[:],
                        q_ro[:, s, :, HD - SHD:HD].rearrange("p h d -> p d h"),
                        axis=AX.X, op=Add,
                    )
                    plm = p2w.tile([P, SHD], F32R, tag="plmq")
                    nc.vector.tensor_scalar(
                        plm[:], pl[:], 1.0 / H, 0.0, op0=Mult, op1=Add
                    )
                    nc.vector.tensor_copy(
                        q_ro[:, s, :, HD - SHD:HD],
                        plm[:].unsqueeze(1).broadcast_to((P, H, SHD)),
                    )
                    gps = p2ps.tile([P, H], FP32, tag="gate")
                    nc.tensor.matmul(
                        gps[:], xq_sb[0:GATE_IN, 0, s * P:(s + 1) * P], wgT_sb[:],
                        start=True, stop=True,
                    )
                    nc.scalar.activation(gate2[:, s, :], gps[:], AF.Sigmoid)
                    nc.vector.tensor_scalar(
                        gate2[:, s, :], gate2[:, s, :], 2.0, 0.0, op0=Mult, op1=Add
                    )
                    for h in range(H):
                        tp = p2tr.tile([P, P], F32R, tag="tp")
                        nc.tensor.transpose(tp[:], q_ro[:, s, h, :], ident[:])
                        pair, off = h // 2, s * 256 + (h % 2) * P
                        nc.scalar.activation(
                            qT[:, pair, off:off + P], tp[:], AF.Copy, scale=ISQ
                        )

            if phase_limit < 4:
                continue
            # ---- Phase 3: attention per kv-group ----
            with (
                tc.tile_pool(name="msk", bufs=1) as msk,
                tc.tile_pool(name="exw", bufs=6) as exw,
                tc.tile_pool(name="p3w", bufs=2) as p3w,
                tc.tile_pool(name="stps", bufs=2, space="PSUM") as stps,
                tc.tile_pool(name="ytps", bufs=2, space="PSUM") as ytps,
                tc.tile_pool(name="dnps", bufs=2, space="PSUM") as dnps,
                tc.tile_pool(name="p3tr", bufs=1, space="PSUM") as p3tr,
            ):
                mask_sb = msk.tile([P, NBLK, 512], F32R)
                nc.sync.dma_start(mask_sb[:], mask_d.ap().rearrange("t p n -> p t n"))
                for g in range(KVH):
                    pairs = (2 * g, 2 * g + 1)
                    yt_ps = {pi: ytps.tile([P, 512], FP32, name="ytp", tag="ytp") for pi in pairs}
                    den_ps = {pi: dnps.tile([1, 512], FP32, name="dnp", tag="dnp") for pi in pairs}
                    for t in range(NBLK):
                        w = 512 if t < NB_B else 256
                        for pi in pairs:
                            st = stps.tile([P, 512], FP32, tag="st")
                            nc.tensor.matmul(
                                st[:, 0:w], kT[:, g, t * P:(t + 1) * P],
                                qT[:, pi, 0:w], start=True, stop=False,
                            )
                            nc.tensor.matmul(
                                st[:, 0:w], ident[:], mask_sb[:, t, 0:w],
                                start=False, stop=True,
                            )
                            ex = exw.tile([P, 512], F32R, tag="ex")
                            nc.scalar.activation(ex[:, 0:w], st[:, 0:w], AF.Exp)
                            if p3sub < 2:
                                continue
                            nc.tensor.matmul(
                                den_ps[pi][:, 0:w], ones_col[:], ex[:, 0:w],
                                start=(t == 0), stop=(t == NBLK - 1),
                                skip_group_check=True,
                            )
                            nc.tensor.matmul(
                                yt_ps[pi][:, 0:w],
                                v_nat[:, t, g * HD:(g + 1) * HD], ex[:, 0:w],
                                start=(t == 0), stop=(t == NBLK - 1),
                                skip_group_check=True,
                            )
                    if p3sub < 3:
                        continue
                    for pi in pairs:
                        den_sb = p3w.tile([1, 512], FP32, tag="densb")
                        nc.vector.tensor_copy(den_sb[:], den_ps[pi][:])
                        dtp = p3tr.tile([P, 4], FP32, tag="dtp")
                        for sub in range(4):
                            nc.tensor.matmul(
                                dtp[:, sub:sub + 1],
                                den_sb[0:1, sub * P:(sub + 1) * P], one11[:],
                                start=True, stop=True, skip_group_check=True,
                            )
                        rdn = p3w.tile([P, 4], FP32, tag="rdn")
                        nc.vector.reciprocal(rdn[:], dtp[:])
                        sc4 = p3w.tile([P, 4], FP32, tag="sc4")
                        nc.vector.tensor_tensor(
                            sc4[:], rdn[:], gate2[:, :, 2 * pi:2 * pi + 2], op=Mult
                        )
                        if p3sub < 4:
                            continue
                        yt_sb = p3w.tile([P, 512], F32R, tag="ytsb")
                        nc.vector.tensor_copy(yt_sb[:], yt_ps[pi][:])
                        for sub in range(4):
                            slot, h = sub // 2, 2 * pi + (sub % 2)
                            typs = p3tr.tile([P, P], F32R, tag="typs")
                            nc.tensor.transpose(
                                typs[:], yt_sb[:, sub * P:(sub + 1) * P], ident[:]
                            )
                            nc.scalar.activation(
                                y_nat[:, slot, h, :], typs[:], AF.Copy,
                                scale=sc4[:, sub:sub + 1],
                            )
                    if p3sub < 5:
                        continue
                    for s in range(2):
                        vg = v_own[:, s, g * HD:(g + 1) * HD]
                        scr = p3w.tile([P, HD], FP32, tag="scr")
                        nc.vector.tensor_tensor(scr[:], vg, vg, op=Mult)
                        n2 = p3w.tile([P, 1], FP32, tag="n2")
                        nc.vector.tensor_reduce(n2[:], scr[:], axis=AX.X, op=Add)
                        nc.vector.tensor_scalar(
                            n2[:], n2[:], 1.0, 1e-24, op0=Mult, op1=Add
                        )
                        rcp2 = p3w.tile([P, 1], FP32, tag="rcp2")
                        nc.vector.reciprocal(rcp2[:], n2[:])
                        rcp2n = p3w.tile([P, 1], FP32, tag="rcp2n")
                        nc.vector.tensor_scalar(
                            rcp2n[:], rcp2[:], -1.0, 0.0, op0=Mult, op1=Add
                        )
                        for h in range(4 * g, 4 * g + 4):
                            yf = y_nat[:, s, h, :].bitcast(FP32)
                            scr2 = p3w.tile([P, HD], FP32, tag="scr2")
                            nc.vector.tensor_tensor(scr2[:], yf, vg, op=Mult)
                            dot = p3w.tile([P, 1], FP32, tag="dot")
                            nc.vector.tensor_reduce(
                                dot[:], scr2[:], axis=AX.X, op=Add
                            )
                            cfn = p3w.tile([P, 1], FP32, tag="cfn")
                            nc.vector.tensor_tensor(
                                cfn[:], dot[:], rcp2n[:], op=Mult
                            )
                            vc = p3w.tile([P, HD], FP32, tag="vc")
                            nc.scalar.activation(
                                vc[:], vg, AF.Copy, scale=cfn[:]
                            )
                            yc = p3w.tile([P, HD], FP32, tag="yc")
                            nc.vector.tensor_tensor(yc[:], yf, vc[:], op=Add)
                            nc.vector.tensor_copy(y_nat[:, s, h, :], yc[:])
            # ---- Phase 4: output projection ----
            with (
                tc.tile_pool(name="wps", bufs=8) as wps,
                tc.tile_pool(name="ytf", bufs=1) as ytf,
                tc.tile_pool(name="p4w", bufs=2) as p4w,
                tc.tile_pool(name="p4ps", bufs=2, space="PSUM") as p4ps,
                tc.tile_pool(name="p4tr", bufs=2, space="PSUM") as p4tr,
            ):
                yT_fin = ytf.tile([P, H, 2 * P], F32R, tag="ytfin")
                for s in range(2):
                    for h in range(H):
                        tp = p4tr.tile([P, P], F32R, tag="tp")
                        nc.tensor.transpose(tp[:], y_nat[:, s, h, :], ident[:])
                        nc.vector.tensor_copy(yT_fin[:, h, s * P:(s + 1) * P], tp[:])
                for ch in range(4):
                    wp_t = []
                    for t in range(NBLK):
                        wt = wps.tile([P, 512], F32R, tag="wp")
                        nc.sync.dma_start(
                            wt[:], wpT_d[t * P:(t + 1) * P, ch * 512:(ch + 1) * 512]
                        )
                        wp_t.append(wt)
                    for s in range(2):
                        ops = p4ps.tile([P, 512], FP32, tag="ops")
                        for t in range(NBLK):
                            nc.tensor.matmul(
                                ops[:], yT_fin[:, t, s * P:(s + 1) * P], wp_t[t][:],
                                start=(t == 0), stop=(t == NBLK - 1),
                            )
                        o_sb = p4w.tile([P, 512], FP32, tag="osb")
                        nc.vector.tensor_copy(o_sb[:], ops[:])
                        nc.sync.dma_start(
                            out_d[s * P:(s + 1) * P, ch * 512:(ch + 1) * 512], o_sb[:]
                        )
    nc.compile()
    return nc


_prog_cache = {}


def _get_program(repeat: int):
    if repeat not in _prog_cache:
        _prog_cache[repeat] = build_program(repeat)
    return _prog_cache[repeat]


def _host_prep(x, cos, sin, Wq, Wk, Wv, Wproj, q_gain, Wg):
    x2 = np.ascontiguousarray(np.asarray(x, np.float32).reshape(S, DIM))
    xT = round_fp32r(x2.T)
    wkT = round_fp32r(np.asarray(Wk, np.float32).T)
    wvT = round_fp32r(np.asarray(Wv, np.float32).T)
    wqT = round_fp32r(np.asarray(Wq, np.float32).T)
    wpT = round_fp32r(np.asarray(Wproj, np.float32).T)
    wgT = round_fp32r(np.asarray(Wg, np.float32).T)
    cosf = np.ascontiguousarray(np.asarray(cos, np.float32).reshape(S, HF))
    sinf = np.ascontiguousarray(np.asarray(sin, np.float32).reshape(S, HF))
    gains = np.ascontiguousarray(
        np.asarray(q_gain, np.float32).reshape(1, H)
    )

    tri_keep = np.triu(np.ones((P, P), bool))  # keep where col >= row
    NEG = np.float32(-1e30)

    in_maps = []
    for c in range(NCORES):
        a, b = 15 - c, c
        rowsA = slice(a * P, (a + 1) * P)
        rowsB = slice(b * P, (b + 1) * P)
        xq = np.concatenate([xT[:, rowsA], xT[:, rowsB]], axis=1)
        cosq = np.concatenate([cosf[rowsA], cosf[rowsB]], axis=0)
        sinq = np.concatenate([sinf[rowsA], sinf[rowsB]], axis=0)
        mask = np.zeros((NBLK, P, 512), np.float32)
        for t in range(NBLK):
            mA = np.zeros((P, P), np.float32)
            if t == a:
                mA[~tri_keep] = NEG
            elif t > a:
                mA[:] = NEG
            mB = np.zeros((P, P), np.float32)
            if t == b:
                mB[~tri_keep] = NEG
            elif t > b:
                mB[:] = NEG
            mask[t] = np.concatenate([mA, mA, mB, mB], axis=1)
        in_maps.append({
            "xT": xT, "xq": np.ascontiguousarray(xq),
            "wkT": wkT, "wvT": wvT, "wqT": wqT, "wpT": wpT, "wgT": wgT,
            "cosk": cosf, "sink": sinf,
            "cosq": np.ascontiguousarray(cosq),
            "sinq": np.ascontiguousarray(sinq),
            "mask": round_fp32r(mask), "gains": gains,
        })
    return in_maps


def kernel(x, cos, sin, Wq, Wk, Wv, Wproj, q_gain, Wg, _repeat=None):
    repeat = _repeat or int(os.environ.get("KERNEL_REPEAT", "1"))
    nc = _get_program(repeat)
    in_maps = _host_prep(x, cos, sin, Wq, Wk, Wv, Wproj, q_gain, Wg)
    res = run_bass_kernel_spmd(nc, in_maps, list(range(NCORES)))
    out = np.zeros((S, DIM), np.float32)
    for c in range(NCORES):
        r = res.results[c]["out_rows"]
        out[(15 - c) * P:(16 - c) * P] = r[0:P]
        out[c * P:(c + 1) * P] = r[P:2 * P]
    v = np.ascontiguousarray(res.results[0]["v_out"]).reshape(1, S, KVH, HD)
    return out.reshape(1, S, DIM), v
